# revision 1
# baseline (speedup 1.0000x reference)
"""DiT attention block on 8 Trainium2 NeuronCores.

Sharding: batch (2) x head-groups (4 heads each) -> 8 cores.
Each core computes, for its batch b and 4 heads:
    q/k/v projections, RMSNorm+rope on q/k, softmax attention, and its
    partial output projection out_partial^T = Wo_slice^T-contraction.
Host sums the 4 head-group partials per batch and transposes back.

Layouts on device (chosen so every matmul has its contraction dim on
partitions and softmax denominators come out of cheap PE reductions):
    xT      [D, S]   = x[b].T
    wqT/wkT [D, 512] = Wq/Wk row-slice transposed (matmul lhsT layout)
    wvT     [D, 512] = Wv row-slice transposed (matmul rhs layout)
    woT     [512, D] = Wo column-slice transposed (lhsT tiles)
    q/k produced transposed per head: [d=128, S]; v in [S, 512].
    scores computed transposed ([k, q]) so exp(scores) feeds the A@V
    matmul directly as the moving operand.
"""

import math

import ml_dtypes
import numpy as np

import concourse.bass as bass
import concourse.mybir as mybir
import concourse.tile as tile
from concourse.bass_utils import run_bass_kernel_spmd

F32 = mybir.dt.float32
F16 = mybir.dt.float16    # matmul operands where precision matters
BF16 = mybir.dt.bfloat16  # wide-range half dtype (exp outputs, V)
P = 128          # partitions / head_dim
S = 2048         # sequence
D = 2048         # model dim
HD = 128         # head dim
NH = 16          # total heads
NHL = 4          # heads per core
IL = NHL * HD    # 512, inner slice per core
KO = D // P      # 16 contraction tiles
SC = 512         # x-chunk columns in the QKV phase
NSC = S // SC    # 8
QC = 512         # q-chunk columns in attention / out phases
NQC = S // QC    # 4
EPS = 1e-6
SCALE = 1.0 / math.sqrt(HD)
N_CORES = 8

_PROG_CACHE = {}


def _split_multi_waits(nc, max_waits=1):
    """walrus here rejects >1 sync-wait per instruction; move extras onto
    same-engine nops placed immediately before the instruction."""
    n_split = 0
    for fn in nc.m.functions:
        for bb in fn.blocks:
            insts = bb.instructions
            new_list = []
            changed = False
            for inst in insts:
                si = getattr(inst, "sync_info", None)
                waits = list(si.on_wait) if (si is not None and si.on_wait) else []
                if len(waits) > max_waits:
                    extra = waits[:-max_waits]
                    keep = waits[-max_waits:]
                    for i in range(0, len(extra), max_waits):
                        nop = mybir.InstNoOp(
                            name=f"I-wsplit-{nc.next_id()}", ins=[], outs=[])
                        nop.engine = inst.engine
                        nop.sync_info = mybir.SyncInfo(
                            on_wait=extra[i:i + max_waits], on_update=[])
                        new_list.append(nop)
                        n_split += 1
                    del si.on_wait[:]
                    si.on_wait.extend(keep)
                    changed = True
                new_list.append(inst)
            if changed:
                del insts[:]
                insts.extend(new_list)
    return n_split


def _emit(nc, tc, t, phases=(1, 2, 3)):
    from contextlib import ExitStack

    Act = mybir.ActivationFunctionType

    with ExitStack() as top:
        top.enter_context(nc.allow_low_precision(
            reason="fp16/bf16 matmul operands; fp32 kept where it matters"))
        const = top.enter_context(tc.tile_pool(name="const", bufs=1))

        rope = {}
        for nm in ("ropeAq", "ropeBq", "ropeAk", "ropeBk"):
            til = const.tile([P, S], F32, tag=nm)
            nc.sync.dma_start(til[:], t[nm][:, :])
            rope[nm] = til
        pswap = const.tile([P, P], F16, tag="pswap")
        nc.sync.dma_start(pswap[:], t["pswap"][:, :])
        ones_col = const.tile([P, 1], F32, tag="ones_col")
        nc.vector.memset(ones_col, 1.0)
        ones_col_hf = const.tile([P, 1], F16, tag="ones_col_hf")
        nc.vector.memset(ones_col_hf, 1.0)
        ones_row_hf = const.tile([1, P], F16, tag="ones_row_hf")
        nc.vector.memset(ones_row_hf, 1.0)
        ones_row = const.tile([1, P], F32, tag="ones_row")
        nc.vector.memset(ones_row, 1.0)
        eps_t = const.tile([1, 1], F32, tag="eps")
        nc.vector.memset(eps_t, EPS)

        # SBUF-resident q/k (transposed per head) and V for the whole kernel
        resid = top.enter_context(tc.tile_pool(name="resid", bufs=1))
        qres = [resid.tile([P, S], F16, tag=f"qres{h}", name=f"qres{h}")
                for h in range(NHL)]
        kres = [resid.tile([P, S], F16, tag=f"kres{h}", name=f"kres{h}")
                for h in range(NHL)]
        vres = resid.tile([P, KO, IL], BF16, tag="vres")

        # ---------------- phase 1: QKV projections + RMSNorm + rope ---------
        if 1 in phases:
          with ExitStack() as ph:
            wpool = ph.enter_context(tc.tile_pool(name="w", bufs=1))
            xpool = ph.enter_context(tc.tile_pool(name="x", bufs=2))
            tp = ph.enter_context(tc.tile_pool(name="qkvtmp", bufs=3))
            ps = ph.enter_context(tc.tile_pool(name="ps_qkv", bufs=3, space="PSUM"))
            ps_sw = ph.enter_context(tc.tile_pool(name="ps_sw", bufs=2, space="PSUM"))
            ps_pb = ph.enter_context(tc.tile_pool(name="ps_pb", bufs=2, space="PSUM"))
            ps1 = ph.enter_context(tc.tile_pool(name="ps1_qkv", bufs=1, space="PSUM"))

            wq = wpool.tile([P, KO, IL], F16, tag="wq")
            nc.sync.dma_start(wq[:], t["wqT"][:, :].rearrange("(ko p) i -> p ko i", p=P))
            wk = wpool.tile([P, KO, IL], F16, tag="wk")
            nc.sync.dma_start(wk[:], t["wkT"][:, :].rearrange("(ko p) i -> p ko i", p=P))
            wv = wpool.tile([P, KO, IL], F16, tag="wv")
            nc.sync.dma_start(wv[:], t["wvT"][:, :].rearrange("(ko p) i -> p ko i", p=P))

            xT_r = t["xT"][:, :].rearrange("(ko p) s -> p ko s", p=P)

            for sc in range(NSC):
                xt = xpool.tile([P, KO, SC], F16, tag="xchunk")
                nc.sync.dma_start(xt[:], xT_r[:, :, sc * SC:(sc + 1) * SC])

                # V projection ([s, i] layout, lhsT = x subtiles)
                for st in range(SC // P):
                    pv = ps.tile([P, QC], F32, tag="ps_main")
                    for kk in range(KO):
                        nc.tensor.matmul(
                            pv[:, :],
                            lhsT=xt[:, kk, st * P:(st + 1) * P],
                            rhs=wv[:, kk, :],
                            start=(kk == 0), stop=(kk == KO - 1))
                    nc.scalar.copy(vres[:, sc * (SC // P) + st, :], pv[:, :])

                # Q / K projections in transposed layout + norm + rope
                for wt, ra, rb, dst in (
                    (wq, rope["ropeAq"], rope["ropeBq"], qres),
                    (wk, rope["ropeAk"], rope["ropeBk"], kres),
                ):
                    for h in range(NHL):
                        pqk = ps.tile([P, QC], F32, tag="ps_main")
                        for kk in range(KO):
                            nc.tensor.matmul(
                                pqk[:, :SC],
                                lhsT=wt[:, kk, h * P:(h + 1) * P],
                                rhs=xt[:, kk, :],
                                start=(kk == 0), stop=(kk == KO - 1))
                        raw = tp.tile([P, SC], F16, tag="raw")
                        nc.scalar.copy(raw[:], pqk[:, :SC])
                        # sum of squares over head_dim (partitions) on PE
                        sq = tp.tile([P, SC], F16, tag="sq")
                        nc.vector.tensor_mul(sq[:], raw[:], raw[:])
                        pssq = ps1.tile([1, QC], F32, tag="ps_ssq")
                        nc.tensor.matmul(pssq[:, :SC], lhsT=ones_col_hf[:], rhs=sq[:],
                                         start=True, stop=True)
                        srt = tp.tile([1, SC], F32, tag="srt")
                        nc.scalar.activation(srt[:], pssq[:, :SC], func=Act.Sqrt,
                                             bias=eps_t[:], scale=1.0 / HD)
                        rstd = tp.tile([1, SC], F16, tag="rstd")
                        nc.vector.reciprocal(rstd[:], srt[:])
                        # rope: rotate-half swap via permutation matmul
                        psw = ps_sw.tile([P, QC], F32, tag="ps_swt")
                        nc.tensor.matmul(psw[:, :SC], lhsT=pswap[:], rhs=raw[:],
                                         start=True, stop=True)
                        tmp = tp.tile([P, SC], F32, tag="ropetmp")
                        nc.vector.tensor_mul(
                            tmp[:], ra[:, sc * SC:(sc + 1) * SC], raw[:])
                        tmp2 = tp.tile([P, SC], F32, tag="ropetmp2")
                        nc.vector.tensor_mul(
                            tmp2[:], rb[:, sc * SC:(sc + 1) * SC], psw[:, :SC])
                        nc.vector.tensor_add(tmp[:], tmp[:], tmp2[:])
                        # apply rstd (broadcast over partitions via K=1 matmul)
                        pb = ps_pb.tile([P, QC], F32, tag="ps_pbt")
                        nc.tensor.matmul(pb[:, :SC], lhsT=ones_row_hf[:], rhs=rstd[:],
                                         start=True, stop=True)
                        nc.vector.tensor_mul(
                            dst[h][:, sc * SC:(sc + 1) * SC], tmp[:], pb[:, :SC])

        # ---------------- phase 2+3: attention, then output projection ------
        if 2 in phases or 3 in phases:
          with ExitStack() as ph:
            avnp = ph.enter_context(tc.tile_pool(name="avn", bufs=1))
            avn = [avnp.tile([P, S], F16, tag=f"avn{h}", name=f"avn{h}")
                   for h in range(NHL)]

            if 2 in phases:
              with ExitStack() as ap_:
                apool = ap_.enter_context(tc.tile_pool(name="attnT", bufs=3))
                accp = ap_.enter_context(tc.tile_pool(name="acc", bufs=4))
                smt = ap_.enter_context(tc.tile_pool(name="smallt", bufs=2))
                ps_av = ap_.enter_context(
                    tc.tile_pool(name="ps_av", bufs=4, space="PSUM"))
                ps_sc = ap_.enter_context(
                    tc.tile_pool(name="ps_sc", bufs=3, space="PSUM"))
                ps_rs = ap_.enter_context(
                    tc.tile_pool(name="ps_rs", bufs=1, space="PSUM"))

                for h in range(NHL):
                    qn_h = qres[h]
                    kn_h = kres[h]

                    pav = [ps_av.tile([P, QC], F32, tag="ps_avt", name=f"pav{i}")
                           for i in range(NQC)]
                    acc0 = accp.tile([P, S], F32, tag="acc0")
                    acc1 = accp.tile([P, S], F32, tag="acc1")

                    for kt in range(KO):
                        at = apool.tile([P, S], BF16, tag="at")
                        for qc in range(NQC):
                            psc = ps_sc.tile([P, QC], F32, tag="ps_sct")
                            nc.tensor.matmul(
                                psc[:],
                                lhsT=kn_h[:, kt * P:(kt + 1) * P],
                                rhs=qn_h[:, qc * QC:(qc + 1) * QC],
                                start=True, stop=True)
                            nc.scalar.activation(
                                at[:, qc * QC:(qc + 1) * QC], psc[:],
                                func=Act.Exp, scale=SCALE)
                            nc.tensor.matmul(
                                pav[qc][:],
                                lhsT=vres[:, kt, h * HD:(h + 1) * HD],
                                rhs=at[:, qc * QC:(qc + 1) * QC],
                                start=(kt == 0), stop=(kt == KO - 1))
                        # running rowsum accumulators, split across DVE/GpSimd
                        eng, accx = ((nc.vector, acc0) if kt % 2 == 0
                                     else (nc.gpsimd, acc1))
                        if kt < 2:
                            eng.tensor_copy(accx[:], at[:])
                        else:
                            eng.tensor_add(accx[:], accx[:], at[:])
                    nc.vector.tensor_add(acc0[:], acc0[:], acc1[:])

                    # softmax denominator: partition-sum of acc, recip, bcast
                    for qc in range(NQC):
                        prs = ps_rs.tile([1, QC], F32, tag="ps_rst")
                        nc.tensor.matmul(prs[:], lhsT=ones_col[:],
                                         rhs=acc0[:, qc * QC:(qc + 1) * QC],
                                         start=True, stop=True)
                        rs = smt.tile([1, QC], F32, tag="rs")
                        nc.vector.reciprocal(rs[:], prs[:])
                        prb = ps_sc.tile([P, QC], F32, tag="ps_sct")
                        nc.tensor.matmul(prb[:], lhsT=ones_row[:], rhs=rs[:],
                                         start=True, stop=True)
                        rbs = smt.tile([P, QC], F32, tag="rbs")
                        nc.scalar.copy(rbs[:], prb[:])
                        nc.vector.tensor_mul(
                            avn[h][:, qc * QC:(qc + 1) * QC], pav[qc][:], rbs[:])

            # ---------------- output projection -----------------------------
            if 3 in phases:
              with ExitStack() as op_:
                wop = op_.enter_context(tc.tile_pool(name="wo", bufs=3))
                otp = op_.enter_context(tc.tile_pool(name="ot", bufs=4))
                ps_o = op_.enter_context(
                    tc.tile_pool(name="ps_o", bufs=8, space="PSUM"))

                woT_r = t["woT"][:, :].rearrange("(it p) d -> p it d", p=P)
                for dt in range(D // P):
                    wo_t = wop.tile([P, NHL, P], F16, tag="wo_t")
                    nc.sync.dma_start(wo_t[:], woT_r[:, :, dt * P:(dt + 1) * P])
                    po = [ps_o.tile([P, QC], F32, tag="ps_ot", name=f"po{i}")
                          for i in range(NQC)]
                    for it in range(NHL):
                        for qc in range(NQC):
                            nc.tensor.matmul(
                                po[qc][:],
                                lhsT=wo_t[:, it, :],
                                rhs=avn[it][:, qc * QC:(qc + 1) * QC],
                                start=(it == 0), stop=(it == NHL - 1))
                    for qc in range(NQC):
                        ot = otp.tile([P, QC], F32, tag="ot")
                        nc.scalar.copy(ot[:], po[qc][:])
                        nc.sync.dma_start(
                            t["outT"][dt * P:(dt + 1) * P,
                                      qc * QC:(qc + 1) * QC], ot[:])


def _build_program(loop_n=0, phases=(1, 2, 3)):
    key = ("nc", loop_n, tuple(phases))
    if key in _PROG_CACHE:
        return _PROG_CACHE[key]
    nc = bass.Bass()
    t = {}
    t["xT"] = nc.dram_tensor("xT", [D, S], F16, kind="ExternalInput")
    t["wqT"] = nc.dram_tensor("wqT", [D, IL], F16, kind="ExternalInput")
    t["wkT"] = nc.dram_tensor("wkT", [D, IL], F16, kind="ExternalInput")
    t["wvT"] = nc.dram_tensor("wvT", [D, IL], F16, kind="ExternalInput")
    t["woT"] = nc.dram_tensor("woT", [IL, D], F16, kind="ExternalInput")
    for nm in ("ropeAq", "ropeBq", "ropeAk", "ropeBk"):
        t[nm] = nc.dram_tensor(nm, [P, S], F32, kind="ExternalInput")
    t["pswap"] = nc.dram_tensor("pswap", [P, P], F16, kind="ExternalInput")
    t["outT"] = nc.dram_tensor("outT", [D, S], F32, kind="ExternalOutput")

    with tile.TileContext(nc) as tc:
        if loop_n:
            with tc.For_i(0, loop_n):
                _emit(nc, tc, t, phases)
        else:
            _emit(nc, tc, t, phases)
    _split_multi_waits(nc)
    _PROG_CACHE[key] = nc
    return nc


def _prep_in_maps(x, rope_emb, Wq, Wk, Wv, Wo, q_norm_w, k_norm_w):
    x = np.asarray(x, np.float32)
    F = np.asarray(rope_emb, np.float32)[:, 0]          # [S, 64, 2, 2]
    A0 = np.concatenate([F[:, :, 0, 0], F[:, :, 1, 1]], axis=-1)  # [S, 128]
    B0 = np.concatenate([F[:, :, 0, 1], F[:, :, 1, 0]], axis=-1)  # [S, 128]

    def rope_consts(w):
        w = np.asarray(w, np.float32)
        w_sw = np.concatenate([w[64:], w[:64]])
        A = np.ascontiguousarray((A0 * w[None, :]).T)    # [128, S]
        B = np.ascontiguousarray((B0 * w_sw[None, :]).T)
        return A, B

    Aq, Bq = rope_consts(q_norm_w)
    Ak, Bk = rope_consts(k_norm_w)
    pswap = np.zeros((P, P), np.float16)
    for d in range(P):
        pswap[(d + 64) % P, d] = 1.0

    bf = np.float16
    xT = [np.ascontiguousarray(x[b].T).astype(bf) for b in range(x.shape[0])]
    Wq = np.asarray(Wq, np.float32)
    Wk = np.asarray(Wk, np.float32)
    Wv = np.asarray(Wv, np.float32)
    Wo = np.asarray(Wo, np.float32)

    in_maps = []
    for c in range(N_CORES):
        b, hg = divmod(c, NH // NHL)
        sl = slice(hg * IL, (hg + 1) * IL)
        in_maps.append({
            "xT": xT[b],
            "wqT": np.ascontiguousarray(Wq[sl, :].T).astype(bf),
            "wkT": np.ascontiguousarray(Wk[sl, :].T).astype(bf),
            "wvT": np.ascontiguousarray(Wv[sl, :].T).astype(bf),
            "woT": np.ascontiguousarray(Wo[:, sl].T).astype(bf),
            "ropeAq": Aq, "ropeBq": Bq, "ropeAk": Ak, "ropeBk": Bk,
            "pswap": pswap,
        })
    return in_maps


def kernel(x, rope_emb, Wq, Wk, Wv, Wo, q_norm_w, k_norm_w, _trace=False):
    nc = _build_program()
    in_maps = _prep_in_maps(x, rope_emb, Wq, Wk, Wv, Wo, q_norm_w, k_norm_w)
    res = run_bass_kernel_spmd(nc, in_maps, core_ids=list(range(N_CORES)),
                               trace=_trace)
    out = np.empty((2, S, D), np.float32)
    for b in range(2):
        acc = res.results[4 * b]["outT"].copy()
        for hg in range(1, 4):
            acc += res.results[4 * b + hg]["outT"]
        out[b] = acc.T
    if _trace:
        kernel.last_exec_time_ns = res.exec_time_ns
        kernel.last_results = res
    return out



# revision 3
# speedup vs baseline: 1.0328x; 1.0328x over previous
"""DiT attention block on 8 Trainium2 NeuronCores.

Sharding: batch (2) x head-groups (4 heads each) -> 8 cores.
Each core computes, for its batch b and 4 heads:
    q/k/v projections, RMSNorm+rope on q/k, softmax attention, and its
    partial output projection. Host sums the 4 head-group partials per
    batch and transposes back.

v2 changes vs baseline:
  - all-ones [128,128] matmul broadcasts partition-reductions directly
    (RMS sum-of-squares and softmax denominator land pre-broadcast in
    PSUM), removing [1,N] reciprocal/bcast chains.
  - rstd = exp(-0.5*ln(ms+eps)) on ScalarE: one activation table
    (natural_log_exp) for the entire kernel, no Sqrt table switch.
  - reciprocal_approx_fast for softmax denominators (the exact
    iterative `reciprocal` measured ~5us per call).
  - exp over [128,1024] two-bank PSUM tiles (amortizes the ~352-cycle
    ACT instruction overhead).
  - softmax row-sum accumulated in f16 on DVE (plus a little GpSimd),
    with exp biased by -4*ln2 so f16 sums cannot overflow; the bias
    cancels exactly in the normalization.
  - PSUM->SBUF evacuations split across ScalarE/VectorE.
"""

import math

import ml_dtypes
import numpy as np

import concourse.bass as bass
import concourse.mybir as mybir
import concourse.tile as tile
from concourse.bass_utils import run_bass_kernel_spmd

F32 = mybir.dt.float32
F16 = mybir.dt.float16
BF16 = mybir.dt.bfloat16
P = 128          # partitions / head_dim
S = 2048         # sequence
D = 2048         # model dim
HD = 128         # head dim
NH = 16          # total heads
NHL = 4          # heads per core
IL = NHL * HD    # 512, inner slice per core
KO = D // P      # 16 contraction tiles
SC = 512         # x-chunk columns in the QKV phase
NSC = S // SC    # 4
QB = 1024        # q-block in attention phase
NQB = S // QB    # 2
QC = 512         # q-chunk in output projection
NQC = S // QC    # 4
EPS = 1e-6
SCALE = 1.0 / math.sqrt(HD)
EXP_BIAS = -4.0 * math.log(2.0)   # exp(x + b); cancels in normalization
N_CORES = 8

Act = mybir.ActivationFunctionType

_PROG_CACHE = {}


def _split_multi_waits(nc, max_waits=1):
    """walrus here rejects >1 sync-wait per instruction; move extras onto
    same-engine nops placed immediately before the instruction."""
    n_split = 0
    for fn in nc.m.functions:
        for bb in fn.blocks:
            insts = bb.instructions
            new_list = []
            changed = False
            for inst in insts:
                si = getattr(inst, "sync_info", None)
                waits = list(si.on_wait) if (si is not None and si.on_wait) else []
                if len(waits) > max_waits:
                    extra = waits[:-max_waits]
                    keep = waits[-max_waits:]
                    for i in range(0, len(extra), max_waits):
                        nop = mybir.InstNoOp(
                            name=f"I-wsplit-{nc.next_id()}", ins=[], outs=[])
                        nop.engine = inst.engine
                        nop.sync_info = mybir.SyncInfo(
                            on_wait=extra[i:i + max_waits], on_update=[])
                        new_list.append(nop)
                        n_split += 1
                    del si.on_wait[:]
                    si.on_wait.extend(keep)
                    changed = True
                new_list.append(inst)
            if changed:
                del insts[:]
                insts.extend(new_list)
    return n_split


def _emit(nc, tc, t, phases=(1, 2, 3)):
    from contextlib import ExitStack

    with ExitStack() as top:
        top.enter_context(nc.allow_low_precision(
            reason="f16 operands; fp32 accumulation where it matters"))
        const = top.enter_context(tc.tile_pool(name="const", bufs=1))

        rope = {}
        for nm in ("ropeAq", "ropeBq", "ropeAk", "ropeBk"):
            til = const.tile([P, S], F16, tag=nm)
            nc.sync.dma_start(til[:], t[nm][:, :])
            rope[nm] = til
        pswap = const.tile([P, P], F16, tag="pswap")
        nc.sync.dma_start(pswap[:], t["pswap"][:, :])
        ones_mat = const.tile([P, P], F16, tag="ones_mat")
        nc.vector.memset(ones_mat, 1.0)

        # SBUF-resident q/k (transposed per head), V, and attn output
        resid = top.enter_context(tc.tile_pool(name="resid", bufs=1))
        qres = [resid.tile([P, S], F16, tag=f"qres{h}", name=f"qres{h}")
                for h in range(NHL)]
        kres = [resid.tile([P, S], F16, tag=f"kres{h}", name=f"kres{h}")
                for h in range(NHL)]
        vres = resid.tile([P, KO, IL], F16, tag="vres")
        avn = [resid.tile([P, S], F16, tag=f"avn{h}", name=f"avn{h}")
               for h in range(NHL)]

        # ---------------- phase 1: QKV projections + RMSNorm + rope ---------
        if 1 in phases:
          with ExitStack() as ph:
            wpool = ph.enter_context(tc.tile_pool(name="w", bufs=1))
            xpool = ph.enter_context(tc.tile_pool(name="x", bufs=2))
            tp = ph.enter_context(tc.tile_pool(name="qkvtmp", bufs=3))
            ps = ph.enter_context(tc.tile_pool(name="ps_qkv", bufs=3, space="PSUM"))
            ps_sw = ph.enter_context(tc.tile_pool(name="ps_sw", bufs=2, space="PSUM"))
            ps_sb = ph.enter_context(tc.tile_pool(name="ps_sb", bufs=2, space="PSUM"))

            wq = wpool.tile([P, KO, IL], F16, tag="wq")
            nc.sync.dma_start(wq[:], t["wqT"][:, :].rearrange("(ko p) i -> p ko i", p=P))
            wk = wpool.tile([P, KO, IL], F16, tag="wk")
            nc.sync.dma_start(wk[:], t["wkT"][:, :].rearrange("(ko p) i -> p ko i", p=P))
            wv = wpool.tile([P, KO, IL], F16, tag="wv")
            nc.sync.dma_start(wv[:], t["wvT"][:, :].rearrange("(ko p) i -> p ko i", p=P))

            xT_r = t["xT"][:, :].rearrange("(ko p) s -> p ko s", p=P)

            for sc in range(NSC):
                xt = xpool.tile([P, KO, SC], F16, tag="xchunk")
                nc.sync.dma_start(xt[:], xT_r[:, :, sc * SC:(sc + 1) * SC])

                # V projection ([s, i] layout, lhsT = x subtiles)
                for st in range(SC // P):
                    pv = ps.tile([P, SC], F32, tag="ps_main")
                    for kk in range(KO):
                        nc.tensor.matmul(
                            pv[:, :],
                            lhsT=xt[:, kk, st * P:(st + 1) * P],
                            rhs=wv[:, kk, :],
                            start=(kk == 0), stop=(kk == KO - 1))
                    nc.scalar.copy(vres[:, sc * (SC // P) + st, :], pv[:, :])

                # Q / K projections in transposed layout + norm + rope
                for wt, ra, rb, dst in (
                    (wq, rope["ropeAq"], rope["ropeBq"], qres),
                    (wk, rope["ropeAk"], rope["ropeBk"], kres),
                ):
                    for h in range(NHL):
                        pqk = ps.tile([P, SC], F32, tag="ps_main")
                        for kk in range(KO):
                            nc.tensor.matmul(
                                pqk[:, :],
                                lhsT=wt[:, kk, h * P:(h + 1) * P],
                                rhs=xt[:, kk, :],
                                start=(kk == 0), stop=(kk == KO - 1))
                        raw = tp.tile([P, SC], F16, tag="raw")
                        nc.scalar.copy(raw[:], pqk[:])
                        # sum of squares over head_dim -> broadcast in one
                        # all-ones matmul; rstd = exp(-0.5*ln(ms+eps))
                        sq = tp.tile([P, SC], F16, tag="sq")
                        nc.vector.tensor_mul(sq[:], raw[:], raw[:])
                        pssq = ps_sb.tile([P, SC], F32, tag="ps_ssq")
                        nc.tensor.matmul(pssq[:], lhsT=ones_mat[:], rhs=sq[:],
                                         start=True, stop=True)
                        lt = tp.tile([P, SC], F32, tag="lnms")
                        nc.scalar.activation(lt[:], pssq[:], func=Act.Ln,
                                             bias=EPS, scale=1.0 / HD)
                        rstd = tp.tile([P, SC], F16, tag="rstd")
                        nc.scalar.activation(rstd[:], lt[:], func=Act.Exp,
                                             scale=-0.5)
                        # rope: rotate-half swap via permutation matmul
                        psw = ps_sw.tile([P, SC], F32, tag="ps_swt")
                        nc.tensor.matmul(psw[:], lhsT=pswap[:], rhs=raw[:],
                                         start=True, stop=True)
                        tmp = tp.tile([P, SC], F16, tag="ropetmp")
                        nc.vector.tensor_mul(
                            tmp[:], ra[:, sc * SC:(sc + 1) * SC], raw[:])
                        tmp2 = tp.tile([P, SC], F32, tag="ropetmp2")
                        nc.vector.tensor_mul(
                            tmp2[:], rb[:, sc * SC:(sc + 1) * SC], psw[:])
                        roped = tp.tile([P, SC], F16, tag="roped")
                        nc.vector.tensor_add(roped[:], tmp[:], tmp2[:])
                        nc.vector.tensor_mul(
                            dst[h][:, sc * SC:(sc + 1) * SC], roped[:], rstd[:])

        # ---------------- phase 2: attention ------------------------------
        if 2 in phases:
          with ExitStack() as ap_:
            atp = ap_.enter_context(tc.tile_pool(name="attnT", bufs=4))
            accp = ap_.enter_context(tc.tile_pool(name="acc", bufs=4))
            smt = ap_.enter_context(tc.tile_pool(name="smallt", bufs=3))
            ps_av = ap_.enter_context(
                tc.tile_pool(name="ps_av", bufs=2, space="PSUM"))
            ps_sc = ap_.enter_context(
                tc.tile_pool(name="ps_sc", bufs=2, space="PSUM"))

            for h in range(NHL):
                for qb in range(NQB):
                    q0 = qb * QB
                    pav = ps_av.tile([P, QB], F32, tag="ps_avt")
                    acc0 = accp.tile([P, QB], F16, tag="acc0")
                    acc1 = accp.tile([P, QB], F16, tag="acc1")

                    for kt in range(KO):
                        psc = ps_sc.tile([P, QB], F32, tag="ps_sct")
                        for half in range(2):
                            nc.tensor.matmul(
                                psc[:, half * QC:(half + 1) * QC],
                                lhsT=kres[h][:, kt * P:(kt + 1) * P],
                                rhs=qres[h][:, q0 + half * QC:
                                            q0 + (half + 1) * QC],
                                start=True, stop=True)
                        at = atp.tile([P, QB], F16, tag="at")
                        nc.scalar.activation(at[:], psc[:], func=Act.Exp,
                                             bias=EXP_BIAS, scale=SCALE)
                        for half in range(2):
                            nc.tensor.matmul(
                                pav[:, half * QC:(half + 1) * QC],
                                lhsT=vres[:, kt, h * HD:(h + 1) * HD],
                                rhs=at[:, half * QC:(half + 1) * QC],
                                start=(kt == 0), stop=(kt == KO - 1))
                        # running row-sum accumulation (f16, safe w/ EXP_BIAS)
                        if kt == 0:
                            nc.vector.tensor_copy(acc0[:], at[:])
                        elif kt == 1:
                            nc.gpsimd.tensor_copy(acc1[:], at[:])
                        elif kt % 2 == 0:
                            nc.vector.tensor_add(acc0[:], acc0[:], at[:])
                        elif kt in (3, 9, 15):
                            nc.gpsimd.tensor_add(acc1[:], acc1[:], at[:])
                        else:
                            nc.vector.tensor_add(acc1[:], acc1[:], at[:])
                    nc.vector.tensor_add(acc0[:], acc0[:], acc1[:])

                    # denominator: all-ones matmul -> broadcast sum in PSUM,
                    # fast reciprocal, apply to pav
                    for half in range(2):
                        prb = ps_sc.tile([P, QB], F32, tag="ps_sct")
                        nc.tensor.matmul(
                            prb[:, :QC], lhsT=ones_mat[:],
                            rhs=acc0[:, half * QC:(half + 1) * QC],
                            start=True, stop=True)
                        rbs = smt.tile([P, QC], F32, tag="rbs")
                        nc.vector.reciprocal_approx_fast(rbs[:], prb[:, :QC])
                        nc.vector.tensor_mul(
                            avn[h][:, q0 + half * QC:q0 + (half + 1) * QC],
                            pav[:, half * QC:(half + 1) * QC], rbs[:])

        # ---------------- phase 3: output projection ----------------------
        if 3 in phases:
          with ExitStack() as op_:
            wop = op_.enter_context(tc.tile_pool(name="wo", bufs=3))
            otp = op_.enter_context(tc.tile_pool(name="ot", bufs=4))
            ps_o = op_.enter_context(
                tc.tile_pool(name="ps_o", bufs=8, space="PSUM"))

            woT_r = t["woT"][:, :].rearrange("(it p) d -> p it d", p=P)
            for dt in range(D // P):
                wo_t = wop.tile([P, NHL, P], F16, tag="wo_t")
                nc.sync.dma_start(wo_t[:], woT_r[:, :, dt * P:(dt + 1) * P])
                po = [ps_o.tile([P, QC], F32, tag="ps_ot", name=f"po{i}")
                      for i in range(NQC)]
                for it in range(NHL):
                    for qc in range(NQC):
                        nc.tensor.matmul(
                            po[qc][:],
                            lhsT=wo_t[:, it, :],
                            rhs=avn[it][:, qc * QC:(qc + 1) * QC],
                            start=(it == 0), stop=(it == NHL - 1))
                for qc in range(NQC):
                    ot = otp.tile([P, QC], F32, tag="ot")
                    if (dt * NQC + qc) % 2 == 0:
                        nc.scalar.copy(ot[:], po[qc][:])
                    else:
                        nc.vector.tensor_copy(ot[:], po[qc][:])
                    nc.sync.dma_start(
                        t["outT"][dt * P:(dt + 1) * P,
                                  qc * QC:(qc + 1) * QC], ot[:])


def _build_program(loop_n=0, phases=(1, 2, 3)):
    key = ("nc", loop_n, tuple(phases))
    if key in _PROG_CACHE:
        return _PROG_CACHE[key]
    nc = bass.Bass()
    t = {}
    t["xT"] = nc.dram_tensor("xT", [D, S], F16, kind="ExternalInput")
    t["wqT"] = nc.dram_tensor("wqT", [D, IL], F16, kind="ExternalInput")
    t["wkT"] = nc.dram_tensor("wkT", [D, IL], F16, kind="ExternalInput")
    t["wvT"] = nc.dram_tensor("wvT", [D, IL], F16, kind="ExternalInput")
    t["woT"] = nc.dram_tensor("woT", [IL, D], F16, kind="ExternalInput")
    for nm in ("ropeAq", "ropeBq", "ropeAk", "ropeBk"):
        t[nm] = nc.dram_tensor(nm, [P, S], F16, kind="ExternalInput")
    t["pswap"] = nc.dram_tensor("pswap", [P, P], F16, kind="ExternalInput")
    t["outT"] = nc.dram_tensor("outT", [D, S], F32, kind="ExternalOutput")

    with tile.TileContext(nc) as tc:
        if loop_n:
            with tc.For_i(0, loop_n):
                _emit(nc, tc, t, phases)
        else:
            _emit(nc, tc, t, phases)
    _split_multi_waits(nc)
    _PROG_CACHE[key] = nc
    return nc


def _prep_in_maps(x, rope_emb, Wq, Wk, Wv, Wo, q_norm_w, k_norm_w):
    x = np.asarray(x, np.float32)
    F = np.asarray(rope_emb, np.float32)[:, 0]          # [S, 64, 2, 2]
    A0 = np.concatenate([F[:, :, 0, 0], F[:, :, 1, 1]], axis=-1)  # [S, 128]
    B0 = np.concatenate([F[:, :, 0, 1], F[:, :, 1, 0]], axis=-1)  # [S, 128]

    def rope_consts(w):
        w = np.asarray(w, np.float32)
        w_sw = np.concatenate([w[64:], w[:64]])
        A = np.ascontiguousarray((A0 * w[None, :]).T)    # [128, S]
        B = np.ascontiguousarray((B0 * w_sw[None, :]).T)
        return A.astype(np.float16), B.astype(np.float16)

    Aq, Bq = rope_consts(q_norm_w)
    Ak, Bk = rope_consts(k_norm_w)
    pswap = np.zeros((P, P), np.float16)
    for d in range(P):
        pswap[(d + 64) % P, d] = 1.0

    bf = np.float16
    xT = [np.ascontiguousarray(x[b].T).astype(bf) for b in range(x.shape[0])]
    Wq = np.asarray(Wq, np.float32)
    Wk = np.asarray(Wk, np.float32)
    Wv = np.asarray(Wv, np.float32)
    Wo = np.asarray(Wo, np.float32)

    in_maps = []
    for c in range(N_CORES):
        b, hg = divmod(c, NH // NHL)
        sl = slice(hg * IL, (hg + 1) * IL)
        in_maps.append({
            "xT": xT[b],
            "wqT": np.ascontiguousarray(Wq[sl, :].T).astype(bf),
            "wkT": np.ascontiguousarray(Wk[sl, :].T).astype(bf),
            "wvT": np.ascontiguousarray(Wv[sl, :].T).astype(bf),
            "woT": np.ascontiguousarray(Wo[:, sl].T).astype(bf),
            "ropeAq": Aq, "ropeBq": Bq, "ropeAk": Ak, "ropeBk": Bk,
            "pswap": pswap,
        })
    return in_maps


def kernel(x, rope_emb, Wq, Wk, Wv, Wo, q_norm_w, k_norm_w, _trace=False):
    nc = _build_program()
    in_maps = _prep_in_maps(x, rope_emb, Wq, Wk, Wv, Wo, q_norm_w, k_norm_w)
    res = run_bass_kernel_spmd(nc, in_maps, core_ids=list(range(N_CORES)),
                               trace=_trace)
    out = np.empty((2, S, D), np.float32)
    for b in range(2):
        acc = res.results[4 * b]["outT"].copy()
        for hg in range(1, 4):
            acc += res.results[4 * b + hg]["outT"]
        out[b] = acc.T
    if _trace:
        kernel.last_exec_time_ns = res.exec_time_ns
        kernel.last_results = res
    return out


# revision 15
# speedup vs baseline: 1.6971x; 1.6432x over previous
"""DiT attention block on 8 Trainium2 NeuronCores.

Sharding: batch (2) x head-groups (4 heads each) -> 8 cores.
Each core computes, for its batch b and 4 heads:
    q/k/v projections, RMSNorm+rope on q/k, softmax attention, and its
    partial output projection. Host sums the 4 head-group partials per
    batch and transposes back.

Structure (v4):
  - phase 1: QKV projections (f16 matmuls) + RMSNorm + rope, PE-bound.
    RMS sum-of-squares is broadcast across partitions by an all-ones
    [128,128] matmul; rstd = exp(-0.5*ln(ms+eps)) on ScalarE so the
    whole kernel uses one activation table (natural_log_exp).
  - phase 2: attention, q-half-major: for each 1024-wide q block, all
    4 heads. exp over [128,1024] two-bank PSUM tiles. Softmax
    denominator: row-sums accumulated in bf16 on DVE/GpSimd, then an
    all-ones matmul broadcasts the partition sum, 1/den = exp(-ln(den))
    on ScalarE (the exact `reciprocal` costs ~3-5us/call).
  - phase 3 (output projection) for q-block 0 is interleaved into the
    emission of q-block 1's attention, filling TensorE during the
    ACT-bound softmax stretch. Output staged in bf16, one DMA per
    [128, 1024] chunk.
  - fp8 was evaluated and rejected: this problem's rope uses random
    (non-orthogonal) 2x2 mixes, scores reach +-22, and softmax logits
    need <0.3% error; fp8 q/k gives ~10% output error (V-only ~2.9%,
    Wo-only ~3.6%, all past the 2e-2 gate).
"""

import math

import ml_dtypes
import numpy as np

import concourse.bass as bass
import concourse.mybir as mybir
import concourse.tile as tile
from concourse.bass_utils import run_bass_kernel_spmd

F32 = mybir.dt.float32
F16 = mybir.dt.float16
BF16 = mybir.dt.bfloat16
P = 128          # partitions / head_dim
S = 2048         # sequence
D = 2048         # model dim
HD = 128         # head dim
NH = 16          # total heads
NHL = 4          # heads per core
IL = NHL * HD    # 512, inner slice per core
KO = D // P      # 16 contraction tiles
SC = 512         # x-chunk columns in the QKV phase
NSC = S // SC    # 4
QB = 1024        # q-block in attention phase
NQB = S // QB    # 2
QC = 512         # q-chunk in output projection
NQC = QB // QC   # 2 per q-block
NDT = D // P     # 16 output row blocks
EPS = 1e-6
SCALE = 1.0 / math.sqrt(HD)
EXP_BIAS = -4.0 * math.log(2.0)   # exp(x + b); cancels in normalization
N_CORES = 8

Act = mybir.ActivationFunctionType

_PROG_CACHE = {}


def _split_multi_waits(nc, max_waits=1):
    """walrus here rejects >1 sync-wait per instruction; move extras onto
    same-engine nops placed immediately before the instruction."""
    n_split = 0
    for fn in nc.m.functions:
        for bb in fn.blocks:
            insts = bb.instructions
            new_list = []
            changed = False
            for inst in insts:
                si = getattr(inst, "sync_info", None)
                waits = list(si.on_wait) if (si is not None and si.on_wait) else []
                if len(waits) > max_waits:
                    extra = waits[:-max_waits]
                    keep = waits[-max_waits:]
                    for i in range(0, len(extra), max_waits):
                        nop = mybir.InstNoOp(
                            name=f"I-wsplit-{nc.next_id()}", ins=[], outs=[])
                        nop.engine = inst.engine
                        nop.sync_info = mybir.SyncInfo(
                            on_wait=extra[i:i + max_waits], on_update=[])
                        new_list.append(nop)
                        n_split += 1
                    del si.on_wait[:]
                    si.on_wait.extend(keep)
                    changed = True
                new_list.append(inst)
            if changed:
                del insts[:]
                insts.extend(new_list)
    return n_split


def _emit(nc, tc, t, phases=(1, 2, 3)):
    from contextlib import ExitStack

    with ExitStack() as top:
        top.enter_context(nc.allow_low_precision(
            reason="f16/bf16 operands; fp32 accumulation where it matters"))
        const = top.enter_context(tc.tile_pool(name="const", bufs=1))

        pswap = const.tile([P, P], F16, tag="pswap")
        nc.sync.dma_start(pswap[:], t["pswap"][:, :])
        ones_mat = const.tile([P, P], BF16, tag="ones_mat")
        nc.vector.memset(ones_mat, 1.0)
        eps_t = const.tile([P, 1], F32, tag="eps_t")
        nc.vector.memset(eps_t, EPS)
        ebias_t = const.tile([P, 1], F32, tag="ebias_t")
        nc.vector.memset(ebias_t, EXP_BIAS)
        rope = {nm: const.tile([P, S], F16, tag=nm, name=nm)
                for nm in ("ropeAq", "ropeBq", "ropeAk", "ropeBk")}

        # SBUF-resident q/k (transposed per head), V, and attn output
        resid = top.enter_context(tc.tile_pool(name="resid", bufs=1))
        qres = [resid.tile([P, S], F16, tag=f"qres{h}", name=f"qres{h}")
                for h in range(NHL)]
        kres = [resid.tile([P, S], F16, tag=f"kres{h}", name=f"kres{h}")
                for h in range(NHL)]
        vres = resid.tile([P, KO, IL], F16, tag="vres")
        avn = [resid.tile([P, S], F16, tag=f"avn{h}", name=f"avn{h}")
               for h in range(NHL)]

        # ---------------- phase 1: QKV projections + RMSNorm + rope ---------
        if 1 in phases:
          with ExitStack() as ph:
            wpool = ph.enter_context(tc.tile_pool(name="w", bufs=1))
            xpool = ph.enter_context(tc.tile_pool(name="x", bufs=2))
            tp = ph.enter_context(tc.tile_pool(name="qkvtmp", bufs=3))
            ps = ph.enter_context(tc.tile_pool(name="ps_qkv", bufs=3, space="PSUM"))
            ps_sw = ph.enter_context(tc.tile_pool(name="ps_sw", bufs=2, space="PSUM"))
            ps_sb = ph.enter_context(tc.tile_pool(name="ps_sb", bufs=2, space="PSUM"))

            # DMA order matters for startup latency: V weights + first x
            # chunk first (V matmuls are the first PE work), then q/k
            # weights, then rope consts (needed ~25us in).
            wv = wpool.tile([P, KO, IL], F16, tag="wv")
            nc.sync.dma_start(wv[:], t["wvT"][:, :].rearrange("(ko p) i -> p ko i", p=P))
            xT_r = t["xT"][:, :].rearrange("(ko p) s -> p ko s", p=P)
            xt0 = xpool.tile([P, KO, SC], F16, tag="xchunk")
            nc.sync.dma_start(xt0[:], xT_r[:, :, 0:SC])
            wq = wpool.tile([P, KO, IL], F16, tag="wq")
            nc.sync.dma_start(wq[:], t["wqT"][:, :].rearrange("(ko p) i -> p ko i", p=P))
            wk = wpool.tile([P, KO, IL], F16, tag="wk")
            nc.sync.dma_start(wk[:], t["wkT"][:, :].rearrange("(ko p) i -> p ko i", p=P))
            for nm in ("ropeAq", "ropeBq", "ropeAk", "ropeBk"):
                nc.sync.dma_start(rope[nm][:], t[nm][:, :])

            # PE warmup during the initial DMA wait: junk matmuls on the
            # memset ones tile keep the HAM activity window busy so the
            # first real matmuls run at full clock.
            warm = ps_sw.tile([P, SC], F32, tag="ps_swt", name="warm")
            for _ in range(8):
                nc.tensor.matmul(warm[:, :P], lhsT=ones_mat[:],
                                 rhs=ones_mat[:], start=True, stop=True)

            for sc in range(NSC):
                if sc == 0:
                    xt = xt0
                else:
                    xt = xpool.tile([P, KO, SC], F16, tag="xchunk")
                    nc.sync.dma_start(xt[:], xT_r[:, :, sc * SC:(sc + 1) * SC])

                # V projection ([s, i] layout, lhsT = x subtiles)
                for st in range(SC // P):
                    pv = ps.tile([P, SC], F32, tag="ps_main")
                    for kk in range(KO):
                        nc.tensor.matmul(
                            pv[:, :],
                            lhsT=xt[:, kk, st * P:(st + 1) * P],
                            rhs=wv[:, kk, :],
                            start=(kk == 0), stop=(kk == KO - 1))
                    nc.scalar.copy(vres[:, sc * (SC // P) + st, :], pv[:, :])

                # Q / K projections in transposed layout + norm + rope
                for wt, ra, rb, dst in (
                    (wq, rope["ropeAq"], rope["ropeBq"], qres),
                    (wk, rope["ropeAk"], rope["ropeBk"], kres),
                ):
                    for h in range(NHL):
                        pqk = ps.tile([P, SC], F32, tag="ps_main")
                        for kk in range(KO):
                            nc.tensor.matmul(
                                pqk[:, :],
                                lhsT=wt[:, kk, h * P:(h + 1) * P],
                                rhs=xt[:, kk, :],
                                start=(kk == 0), stop=(kk == KO - 1))
                        raw = tp.tile([P, SC], F16, tag="raw")
                        nc.scalar.copy(raw[:], pqk[:])
                        # sum of squares over head_dim -> broadcast in one
                        # all-ones matmul; rstd = exp(-0.5*ln(ms+eps))
                        sq = tp.tile([P, SC], BF16, tag="sq")
                        nc.gpsimd.tensor_mul(sq[:], raw[:], raw[:])
                        pssq = ps_sb.tile([P, SC], F32, tag="ps_ssq")
                        nc.tensor.matmul(pssq[:], lhsT=ones_mat[:], rhs=sq[:],
                                         start=True, stop=True)
                        lt = tp.tile([P, SC], F32, tag="lnms")
                        nc.scalar.activation(lt[:], pssq[:], func=Act.Ln,
                                             bias=eps_t[:], scale=1.0 / HD)
                        rstd = tp.tile([P, SC], F16, tag="rstd")
                        nc.scalar.activation(rstd[:], lt[:], func=Act.Exp,
                                             scale=-0.5)
                        # rope: rotate-half swap via permutation matmul
                        psw = ps_sw.tile([P, SC], F32, tag="ps_swt")
                        nc.tensor.matmul(psw[:], lhsT=pswap[:], rhs=raw[:],
                                         start=True, stop=True)
                        tmp = tp.tile([P, SC], F16, tag="ropetmp")
                        nc.vector.tensor_mul(
                            tmp[:], ra[:, sc * SC:(sc + 1) * SC], raw[:])
                        tmp2 = tp.tile([P, SC], F32, tag="ropetmp2")
                        nc.vector.tensor_mul(
                            tmp2[:], rb[:, sc * SC:(sc + 1) * SC], psw[:])
                        roped = tp.tile([P, SC], F16, tag="roped")
                        nc.vector.tensor_add(roped[:], tmp[:], tmp2[:])
                        nc.vector.tensor_mul(
                            dst[h][:, sc * SC:(sc + 1) * SC], roped[:], rstd[:])

        # ---------------- phase 2+3: attention with interleaved out-proj ----
        if 2 in phases:
          with ExitStack() as ap_:
            atp = ap_.enter_context(tc.tile_pool(name="attnT", bufs=4))
            accp = ap_.enter_context(tc.tile_pool(name="acc", bufs=4))
            smt = ap_.enter_context(tc.tile_pool(name="smallt", bufs=3))
            wop = ap_.enter_context(tc.tile_pool(name="wo", bufs=3))
            otp = ap_.enter_context(tc.tile_pool(name="ot", bufs=3))
            ps_av = ap_.enter_context(
                tc.tile_pool(name="ps_av", bufs=1, space="PSUM"))
            ps_sc = ap_.enter_context(
                tc.tile_pool(name="ps_sc", bufs=2, space="PSUM"))
            ps_o = ap_.enter_context(
                tc.tile_pool(name="ps_o", bufs=2, space="PSUM"))

            woT_r = t["woT"][:, :].rearrange("(it p) d -> p it d", p=P)

            def att_steps(h, qb):
                """Attention for (head h, q block qb); yields after each kt
                step so out-proj chunks can interleave in emission order."""
                q0 = qb * QB
                pav = ps_av.tile([P, QB], F32, tag="ps_avt", name="pav")
                acc0 = accp.tile([P, QB], BF16, tag="acc0", name="acc0")
                acc1 = accp.tile([P, QB], BF16, tag="acc1", name="acc1")

                for kt in range(KO):
                    psc = ps_sc.tile([P, QB], F32, tag="ps_sct", name="psc")
                    for half in range(2):
                        nc.tensor.matmul(
                            psc[:, half * QC:(half + 1) * QC],
                            lhsT=kres[h][:, kt * P:(kt + 1) * P],
                            rhs=qres[h][:, q0 + half * QC:
                                        q0 + (half + 1) * QC],
                            start=True, stop=True)
                    at = atp.tile([P, QB], BF16, tag="at", name="at")
                    nc.scalar.activation(at[:], psc[:], func=Act.Exp,
                                         bias=ebias_t[:], scale=SCALE)
                    for half in range(2):
                        nc.tensor.matmul(
                            pav[:, half * QC:(half + 1) * QC],
                            lhsT=vres[:, kt, h * HD:(h + 1) * HD],
                            rhs=at[:, half * QC:(half + 1) * QC],
                            start=(kt == 0), stop=(kt == KO - 1))
                    # running row-sum accumulation split DVE/GpSimd
                    if kt == 0:
                        nc.vector.tensor_copy(acc0[:], at[:])
                    elif kt == 1:
                        nc.gpsimd.tensor_copy(acc1[:], at[:])
                    elif kt % 2 == 0:
                        nc.vector.tensor_add(acc0[:], acc0[:], at[:])
                    elif kt in (3, 9, 15):
                        nc.gpsimd.tensor_add(acc1[:], acc1[:], at[:])
                    else:
                        nc.vector.tensor_add(acc1[:], acc1[:], at[:])
                    yield
                nc.vector.tensor_add(acc0[:], acc0[:], acc1[:])

                # denominator: all-ones matmul broadcasts the partition sum;
                # 1/den via exp(-ln(den)) (same ACT table as softmax exp)
                for half in range(2):
                    prb = ps_sc.tile([P, QB], F32, tag="ps_sct", name="prb")
                    nc.tensor.matmul(
                        prb[:, :QC], lhsT=ones_mat[:],
                        rhs=acc0[:, half * QC:(half + 1) * QC],
                        start=True, stop=True)
                    ldn = smt.tile([P, QC], F32, tag="ldn")
                    nc.scalar.activation(ldn[:], prb[:, :QC], func=Act.Ln)
                    rbs = smt.tile([P, QC], F32, tag="rbs")
                    nc.scalar.activation(rbs[:], ldn[:], func=Act.Exp,
                                         scale=-1.0)
                    nc.vector.tensor_mul(
                        avn[h][:, q0 + half * QC:q0 + (half + 1) * QC],
                        pav[:, half * QC:(half + 1) * QC], rbs[:])
                yield

            def p3_chunk(dt, qb):
                """Output projection rows [dt*128, (dt+1)*128) for q block
                qb; bf16-staged, one DMA."""
                q0 = qb * QB
                wo_t = wop.tile([P, NHL, P], F16, tag="wo_t", name="wo_t")
                nc.sync.dma_start(wo_t[:], woT_r[:, :, dt * P:(dt + 1) * P])
                ot = otp.tile([P, QB], BF16, tag="ot", name="ot")
                for qc in range(NQC):
                    po = ps_o.tile([P, QC], F32, tag="ps_ot", name="po")
                    for it in range(NHL):
                        nc.tensor.matmul(
                            po[:],
                            lhsT=wo_t[:, it, :],
                            rhs=avn[it][:, q0 + qc * QC:q0 + (qc + 1) * QC],
                            start=(it == 0), stop=(it == NHL - 1))
                    nc.vector.tensor_copy(ot[:, qc * QC:(qc + 1) * QC], po[:])
                nc.sync.dma_start(
                    t["outT"][dt * P:(dt + 1) * P, qb * QB:(qb + 1) * QB],
                    ot[:])

            # q block 0: all heads, nothing to overlap yet
            for h in range(NHL):
                for _ in att_steps(h, 0):
                    pass
            # q block 1: interleave q-block-0 output projection chunks into
            # the ACT-bound attention stream (one dt chunk per 4 kt steps)
            if 3 in phases:
                dt_iter = iter(range(NDT))
                for h in range(NHL):
                    for i, _ in enumerate(att_steps(h, 1)):
                        if i % 4 == 3:
                            dt = next(dt_iter, None)
                            if dt is not None:
                                p3_chunk(dt, 0)
                for dt in dt_iter:
                    p3_chunk(dt, 0)
                # q block 1 output projection (tail)
                for dt in range(NDT):
                    p3_chunk(dt, 1)
            else:
                for h in range(NHL):
                    for _ in att_steps(h, 1):
                        pass


def _build_program(loop_n=0, phases=(1, 2, 3)):
    key = ("nc", loop_n, tuple(phases))
    if key in _PROG_CACHE:
        return _PROG_CACHE[key]
    nc = bass.Bass()
    t = {}
    t["xT"] = nc.dram_tensor("xT", [D, S], F16, kind="ExternalInput")
    t["wqT"] = nc.dram_tensor("wqT", [D, IL], F16, kind="ExternalInput")
    t["wkT"] = nc.dram_tensor("wkT", [D, IL], F16, kind="ExternalInput")
    t["wvT"] = nc.dram_tensor("wvT", [D, IL], F16, kind="ExternalInput")
    t["woT"] = nc.dram_tensor("woT", [IL, D], F16, kind="ExternalInput")
    for nm in ("ropeAq", "ropeBq", "ropeAk", "ropeBk"):
        t[nm] = nc.dram_tensor(nm, [P, S], F16, kind="ExternalInput")
    t["pswap"] = nc.dram_tensor("pswap", [P, P], F16, kind="ExternalInput")
    t["outT"] = nc.dram_tensor("outT", [D, S], BF16, kind="ExternalOutput")

    with tile.TileContext(nc) as tc:
        if loop_n:
            with tc.For_i(0, loop_n):
                _emit(nc, tc, t, phases)
        else:
            _emit(nc, tc, t, phases)
    _split_multi_waits(nc)
    _PROG_CACHE[key] = nc
    return nc


def _prep_in_maps(x, rope_emb, Wq, Wk, Wv, Wo, q_norm_w, k_norm_w):
    x = np.asarray(x, np.float32)
    F = np.asarray(rope_emb, np.float32)[:, 0]          # [S, 64, 2, 2]
    A0 = np.concatenate([F[:, :, 0, 0], F[:, :, 1, 1]], axis=-1)  # [S, 128]
    B0 = np.concatenate([F[:, :, 0, 1], F[:, :, 1, 0]], axis=-1)  # [S, 128]

    def rope_consts(w):
        w = np.asarray(w, np.float32)
        w_sw = np.concatenate([w[64:], w[:64]])
        A = np.ascontiguousarray((A0 * w[None, :]).T)    # [128, S]
        B = np.ascontiguousarray((B0 * w_sw[None, :]).T)
        return A.astype(np.float16), B.astype(np.float16)

    Aq, Bq = rope_consts(q_norm_w)
    Ak, Bk = rope_consts(k_norm_w)
    pswap = np.zeros((P, P), np.float16)
    for d in range(P):
        pswap[(d + 64) % P, d] = 1.0

    bf = np.float16
    xT = [np.ascontiguousarray(x[b].T).astype(bf) for b in range(x.shape[0])]
    Wq = np.asarray(Wq, np.float32)
    Wk = np.asarray(Wk, np.float32)
    Wv = np.asarray(Wv, np.float32)
    Wo = np.asarray(Wo, np.float32)

    in_maps = []
    for c in range(N_CORES):
        b, hg = divmod(c, NH // NHL)
        sl = slice(hg * IL, (hg + 1) * IL)
        in_maps.append({
            "xT": xT[b],
            "wqT": np.ascontiguousarray(Wq[sl, :].T).astype(bf),
            "wkT": np.ascontiguousarray(Wk[sl, :].T).astype(bf),
            "wvT": np.ascontiguousarray(Wv[sl, :].T).astype(bf),
            "woT": np.ascontiguousarray(Wo[:, sl].T).astype(bf),
            "ropeAq": Aq, "ropeBq": Bq, "ropeAk": Ak, "ropeBk": Bk,
            "pswap": pswap,
        })
    return in_maps


def kernel(x, rope_emb, Wq, Wk, Wv, Wo, q_norm_w, k_norm_w, _trace=False):
    nc = _build_program()
    in_maps = _prep_in_maps(x, rope_emb, Wq, Wk, Wv, Wo, q_norm_w, k_norm_w)
    res = run_bass_kernel_spmd(nc, in_maps, core_ids=list(range(N_CORES)),
                               trace=_trace)
    out = np.empty((2, S, D), np.float32)
    for b in range(2):
        acc = res.results[4 * b]["outT"].astype(np.float32)
        for hg in range(1, 4):
            acc += res.results[4 * b + hg]["outT"].astype(np.float32)
        out[b] = acc.T
    if _trace:
        kernel.last_exec_time_ns = res.exec_time_ns
        kernel.last_results = res
    return out


# revision 17
# speedup vs baseline: 1.7444x; 1.0279x over previous
"""DiT attention block on 8 Trainium2 NeuronCores.

Sharding: batch (2) x head-groups (4 heads each) -> 8 cores.
Each core computes, for its batch b and 4 heads:
    q/k/v projections, RMSNorm+rope on q/k, softmax attention, and its
    partial output projection. Host sums the 4 head-group partials per
    batch and transposes back.

Structure (v4):
  - phase 1: QKV projections (f16 matmuls) + RMSNorm + rope, PE-bound.
    RMS sum-of-squares is broadcast across partitions by an all-ones
    [128,128] matmul; rstd = exp(-0.5*ln(ms+eps)) on ScalarE so the
    whole kernel uses one activation table (natural_log_exp).
  - phase 2: attention, q-half-major: for each 1024-wide q block, all
    4 heads. exp over [128,1024] two-bank PSUM tiles. Softmax
    denominator: row-sums accumulated in bf16 on DVE/GpSimd, then an
    all-ones matmul broadcasts the partition sum, 1/den = exp(-ln(den))
    on ScalarE (the exact `reciprocal` costs ~3-5us/call).
  - phase 3 (output projection) for q-block 0 is interleaved into the
    emission of q-block 1's attention, filling TensorE during the
    ACT-bound softmax stretch. Output staged in bf16, one DMA per
    [128, 1024] chunk.
  - fp8 was evaluated and rejected: this problem's rope uses random
    (non-orthogonal) 2x2 mixes, scores reach +-22, and softmax logits
    need <0.3% error; fp8 q/k gives ~10% output error (V-only ~2.9%,
    Wo-only ~3.6%, all past the 2e-2 gate).
"""

import math

import ml_dtypes
import numpy as np

import concourse.bass as bass
import concourse.mybir as mybir
import concourse.tile as tile
from concourse.bass_utils import run_bass_kernel_spmd

F32 = mybir.dt.float32
F16 = mybir.dt.float16
BF16 = mybir.dt.bfloat16
P = 128          # partitions / head_dim
S = 2048         # sequence
D = 2048         # model dim
HD = 128         # head dim
NH = 16          # total heads
NHL = 4          # heads per core
IL = NHL * HD    # 512, inner slice per core
KO = D // P      # 16 contraction tiles
SC = 512         # x-chunk columns in the QKV phase
NSC = S // SC    # 4
QB = 1024        # q-block in attention phase
NQB = S // QB    # 2
QC = 512         # q-chunk in output projection
NQC = QB // QC   # 2 per q-block
NDT = D // P     # 16 output row blocks
EPS = 1e-6
SCALE = 1.0 / math.sqrt(HD)
EXP_BIAS = -4.0 * math.log(2.0)   # exp(x + b); cancels in normalization
N_CORES = 8

Act = mybir.ActivationFunctionType

_PROG_CACHE = {}


def _split_multi_waits(nc, max_waits=1):
    """walrus here rejects >1 sync-wait per instruction; move extras onto
    same-engine nops placed immediately before the instruction."""
    n_split = 0
    for fn in nc.m.functions:
        for bb in fn.blocks:
            insts = bb.instructions
            new_list = []
            changed = False
            for inst in insts:
                si = getattr(inst, "sync_info", None)
                waits = list(si.on_wait) if (si is not None and si.on_wait) else []
                if len(waits) > max_waits:
                    extra = waits[:-max_waits]
                    keep = waits[-max_waits:]
                    for i in range(0, len(extra), max_waits):
                        nop = mybir.InstNoOp(
                            name=f"I-wsplit-{nc.next_id()}", ins=[], outs=[])
                        nop.engine = inst.engine
                        nop.sync_info = mybir.SyncInfo(
                            on_wait=extra[i:i + max_waits], on_update=[])
                        new_list.append(nop)
                        n_split += 1
                    del si.on_wait[:]
                    si.on_wait.extend(keep)
                    changed = True
                new_list.append(inst)
            if changed:
                del insts[:]
                insts.extend(new_list)
    return n_split


def _emit(nc, tc, t, phases=(1, 2, 3)):
    from contextlib import ExitStack

    with ExitStack() as top:
        top.enter_context(nc.allow_low_precision(
            reason="f16/bf16 operands; fp32 accumulation where it matters"))
        const = top.enter_context(tc.tile_pool(name="const", bufs=1))

        pswap = const.tile([P, P], F16, tag="pswap")
        nc.sync.dma_start(pswap[:], t["pswap"][:, :])
        ones_mat = const.tile([P, P], BF16, tag="ones_mat")
        nc.vector.memset(ones_mat, 1.0)
        eps_t = const.tile([P, 1], F32, tag="eps_t")
        nc.vector.memset(eps_t, EPS)
        ebias_t = const.tile([P, 1], F32, tag="ebias_t")
        nc.vector.memset(ebias_t, EXP_BIAS)
        rope = {nm: const.tile([P, S], F16, tag=nm, name=nm)
                for nm in ("ropeAq", "ropeBq", "ropeAk", "ropeBk")}

        # SBUF-resident q/k (transposed per head), V, and attn output
        resid = top.enter_context(tc.tile_pool(name="resid", bufs=1))
        qres = [resid.tile([P, S], F16, tag=f"qres{h}", name=f"qres{h}")
                for h in range(NHL)]
        kres = [resid.tile([P, S], F16, tag=f"kres{h}", name=f"kres{h}")
                for h in range(NHL)]
        vres = resid.tile([P, KO, IL], F16, tag="vres")
        avn = [resid.tile([P, S], F16, tag=f"avn{h}", name=f"avn{h}")
               for h in range(NHL)]

        # ------------------------------------------------------------------
        # Unified emission: phase 1 (QKV+norm+rope), attention, and output
        # projection share one PSUM layout so their instruction streams can
        # interleave:
        #   psA   2 x [P,QB] f32 (4 banks): scores tiles; p1 main
        #         projection accumulators during the dense phase-1 part
        #   psPav 1 x [P,QB] f32 (2 banks): attention AV accumulator
        #   psB   2 x [P,SC] f32 (2 banks): ssq/pswap/denominator/out-proj
        # Emission order:
        #   chunks 0-2: V + all 8 q/k tiles (PE-dense)
        #   chunk 3:    V + the 4 k tiles, then attention on q block 0
        #               cascades in, interleaved with the remaining q tiles
        #   q block 1:  attention interleaved with q-block-0 output chunks
        #   tail:       q-block-1 output chunks
        # ------------------------------------------------------------------
        from contextlib import ExitStack as _ES
        ph = top.enter_context(_ES())
        wpool = ph.enter_context(tc.tile_pool(name="w", bufs=1))
        xpool = ph.enter_context(tc.tile_pool(name="x", bufs=2))
        tp = ph.enter_context(tc.tile_pool(name="qkvtmp", bufs=3))
        tps = ph.enter_context(tc.tile_pool(name="qkvtmps", bufs=2))
        tp2 = ph.enter_context(tc.tile_pool(name="qkvtmp32", bufs=2))
        atp = top.enter_context(tc.tile_pool(name="attnT", bufs=3))
        accp = top.enter_context(tc.tile_pool(name="acc", bufs=2))
        smt = top.enter_context(tc.tile_pool(name="smallt", bufs=2))
        wop = top.enter_context(tc.tile_pool(name="wo", bufs=2))
        otp = top.enter_context(tc.tile_pool(name="ot", bufs=2))
        psA = top.enter_context(tc.tile_pool(name="psA", bufs=2, space="PSUM"))
        psPav = top.enter_context(
            tc.tile_pool(name="psPav", bufs=1, space="PSUM"))
        psB = top.enter_context(tc.tile_pool(name="psB", bufs=2, space="PSUM"))

        # DMA order matters for startup latency: V weights + first x chunk
        # first (V matmuls are the first PE work), split in halves so the
        # first matmuls can start sooner.
        wv = wpool.tile([P, KO, IL], F16, tag="wv")
        wvT_r = t["wvT"][:, :].rearrange("(ko p) i -> p ko i", p=P)
        nc.sync.dma_start(wv[:, 0:KO // 2, :], wvT_r[:, 0:KO // 2, :])
        nc.sync.dma_start(wv[:, KO // 2:, :], wvT_r[:, KO // 2:, :])
        xT_r = t["xT"][:, :].rearrange("(ko p) s -> p ko s", p=P)
        xt0 = xpool.tile([P, KO, SC], F16, tag="xchunk")
        nc.sync.dma_start(xt0[:, 0:KO // 2, :], xT_r[:, 0:KO // 2, 0:SC])
        nc.sync.dma_start(xt0[:, KO // 2:, :], xT_r[:, KO // 2:, 0:SC])
        wq = wpool.tile([P, KO, IL], F16, tag="wq")
        nc.sync.dma_start(wq[:], t["wqT"][:, :].rearrange("(ko p) i -> p ko i", p=P))
        wk = wpool.tile([P, KO, IL], F16, tag="wk")
        nc.sync.dma_start(wk[:], t["wkT"][:, :].rearrange("(ko p) i -> p ko i", p=P))
        for nm in ("ropeAq", "ropeBq", "ropeAk", "ropeBk"):
            nc.sync.dma_start(rope[nm][:], t[nm][:, :])

        # PE warmup during the initial DMA wait (junk matmuls on memset
        # data): gets the HAM clock gate to full rate before real work.
        warm = psB.tile([P, SC], F32, tag="psB", name="warm")
        for _ in range(8):
            nc.tensor.matmul(warm[:, :P], lhsT=ones_mat[:],
                             rhs=ones_mat[:], start=True, stop=True)

        def v_tile(xt, sc, st, pool, tag):
            pv = pool.tile([P, SC], F32, tag=tag, name="pv")
            for kk in range(KO):
                nc.tensor.matmul(
                    pv[:, :],
                    lhsT=xt[:, kk, st * P:(st + 1) * P],
                    rhs=wv[:, kk, :],
                    start=(kk == 0), stop=(kk == KO - 1))
            nc.scalar.copy(vres[:, sc * (SC // P) + st, :], pv[:, :])

        def qk_tile(xt, sc, wt, ra, rb, dst, h, pool, tag):
            pqk = pool.tile([P, SC], F32, tag=tag, name="pqk")
            for kk in range(KO):
                nc.tensor.matmul(
                    pqk[:, :SC],
                    lhsT=wt[:, kk, h * P:(h + 1) * P],
                    rhs=xt[:, kk, :],
                    start=(kk == 0), stop=(kk == KO - 1))
            raw = tp.tile([P, SC], F16, tag="raw")
            nc.scalar.copy(raw[:], pqk[:, :SC])
            # sum of squares over head_dim, broadcast by all-ones matmul;
            # rstd = exp(-0.5*ln(ms+eps)) keeps everything in one ACT table
            sq = tps.tile([P, SC], BF16, tag="sq")
            nc.gpsimd.tensor_mul(sq[:], raw[:], raw[:])
            pssq = psB.tile([P, SC], F32, tag="psB", name="pssq")
            nc.tensor.matmul(pssq[:], lhsT=ones_mat[:], rhs=sq[:],
                             start=True, stop=True)
            lt = tp2.tile([P, SC], F32, tag="lnms")
            nc.scalar.activation(lt[:], pssq[:], func=Act.Ln,
                                 bias=eps_t[:], scale=1.0 / HD)
            rstd = tp.tile([P, SC], F16, tag="rstd")
            nc.scalar.activation(rstd[:], lt[:], func=Act.Exp, scale=-0.5)
            # rope rotate-half swap via permutation matmul
            psw = psB.tile([P, SC], F32, tag="psB", name="psw")
            nc.tensor.matmul(psw[:], lhsT=pswap[:], rhs=raw[:],
                             start=True, stop=True)
            tmp = tps.tile([P, SC], F16, tag="ropetmp")
            nc.vector.tensor_mul(tmp[:], ra[:, sc * SC:(sc + 1) * SC], raw[:])
            tmp2 = tps.tile([P, SC], F16, tag="ropetmp2")
            nc.vector.tensor_mul(tmp2[:], rb[:, sc * SC:(sc + 1) * SC], psw[:])
            roped = tps.tile([P, SC], F16, tag="roped")
            nc.vector.tensor_add(roped[:], tmp[:], tmp2[:])
            nc.vector.tensor_mul(
                dst[h][:, sc * SC:(sc + 1) * SC], roped[:], rstd[:])

        QSET = (wq, rope["ropeAq"], rope["ropeBq"], qres)
        KSET = (wk, rope["ropeAk"], rope["ropeBk"], kres)

        def att_steps(h, qb):
            """Attention for (head h, q block qb); yields after each kt
            step so other work can interleave in emission order."""
            q0 = qb * QB
            pav = psPav.tile([P, QB], F32, tag="psPav", name="pav")
            acc0 = accp.tile([P, QB], BF16, tag="acc0", name="acc0")
            acc1 = accp.tile([P, QB], BF16, tag="acc1", name="acc1")

            for kt in range(KO):
                psc = psA.tile([P, QB], F32, tag="psA", name="psc")
                for half in range(2):
                    nc.tensor.matmul(
                        psc[:, half * QC:(half + 1) * QC],
                        lhsT=kres[h][:, kt * P:(kt + 1) * P],
                        rhs=qres[h][:, q0 + half * QC:q0 + (half + 1) * QC],
                        start=True, stop=True)
                at = atp.tile([P, QB], BF16, tag="at", name="at")
                nc.scalar.activation(at[:], psc[:], func=Act.Exp,
                                     bias=ebias_t[:], scale=SCALE)
                for half in range(2):
                    nc.tensor.matmul(
                        pav[:, half * QC:(half + 1) * QC],
                        lhsT=vres[:, kt, h * HD:(h + 1) * HD],
                        rhs=at[:, half * QC:(half + 1) * QC],
                        start=(kt == 0), stop=(kt == KO - 1))
                # running row-sum accumulation split DVE/GpSimd
                if kt == 0:
                    nc.vector.tensor_copy(acc0[:], at[:])
                elif kt == 1:
                    nc.gpsimd.tensor_copy(acc1[:], at[:])
                elif kt % 2 == 0:
                    nc.vector.tensor_add(acc0[:], acc0[:], at[:])
                elif kt in (3, 9, 15):
                    nc.gpsimd.tensor_add(acc1[:], acc1[:], at[:])
                else:
                    nc.vector.tensor_add(acc1[:], acc1[:], at[:])
                yield

            # denominator: two accumulated all-ones matmuls (acc0 + acc1)
            # broadcast the partition sum; 1/den via exp(-ln(den))
            for half in range(2):
                prb = psB.tile([P, QC], F32, tag="psB", name="prb")
                nc.tensor.matmul(
                    prb[:], lhsT=ones_mat[:],
                    rhs=acc0[:, half * QC:(half + 1) * QC],
                    start=True, stop=False)
                nc.tensor.matmul(
                    prb[:], lhsT=ones_mat[:],
                    rhs=acc1[:, half * QC:(half + 1) * QC],
                    start=False, stop=True)
                ldn = smt.tile([P, QC], F32, tag="ldn")
                nc.scalar.activation(ldn[:], prb[:], func=Act.Ln)
                rbs = smt.tile([P, QC], F32, tag="rbs")
                nc.scalar.activation(rbs[:], ldn[:], func=Act.Exp,
                                     scale=-1.0)
                nc.vector.tensor_mul(
                    avn[h][:, q0 + half * QC:q0 + (half + 1) * QC],
                    pav[:, half * QC:(half + 1) * QC], rbs[:])
            yield

        woT_r = t["woT"][:, :].rearrange("(it p) d -> p it d", p=P)

        def p3_chunk(dt, qb):
            """Output projection rows [dt*128,(dt+1)*128) x q block qb;
            bf16-staged, one DMA."""
            q0 = qb * QB
            wo_t = wop.tile([P, NHL, P], F16, tag="wo_t", name="wo_t")
            nc.sync.dma_start(wo_t[:], woT_r[:, :, dt * P:(dt + 1) * P])
            ot = otp.tile([P, QB], BF16, tag="ot", name="ot")
            for qc in range(NQC):
                po = psB.tile([P, QC], F32, tag="psB", name="po")
                for it in range(NHL):
                    nc.tensor.matmul(
                        po[:],
                        lhsT=wo_t[:, it, :],
                        rhs=avn[it][:, q0 + qc * QC:q0 + (qc + 1) * QC],
                        start=(it == 0), stop=(it == NHL - 1))
                nc.vector.tensor_copy(ot[:, qc * QC:(qc + 1) * QC], po[:])
            nc.sync.dma_start(
                t["outT"][dt * P:(dt + 1) * P, qb * QB:(qb + 1) * QB], ot[:])

        # ---- chunks 0-2: dense phase 1 ----
        for sc in range(NSC - 1):
            if sc == 0:
                xt = xt0
            else:
                xt = xpool.tile([P, KO, SC], F16, tag="xchunk")
                nc.sync.dma_start(xt[:], xT_r[:, :, sc * SC:(sc + 1) * SC])
            for st in range(SC // P):
                v_tile(xt, sc, st, psA, "psA")
            for wt, ra, rb, dst in (QSET, KSET):
                for h in range(NHL):
                    qk_tile(xt, sc, wt, ra, rb, dst, h, psA, "psA")

        # ---- chunk 3: V + k tiles, then q-block-0 attention cascades in,
        # interleaved with the remaining q tiles ----
        sc = NSC - 1
        xt3 = xpool.tile([P, KO, SC], F16, tag="xchunk")
        nc.sync.dma_start(xt3[:], xT_r[:, :, sc * SC:(sc + 1) * SC])
        for st in range(SC // P):
            v_tile(xt3, sc, st, psB, "psB")
        for h in range(NHL):
            qk_tile(xt3, sc, *KSET, h, psB, "psB")

        pending_q = list(range(NHL))
        for h in range(NHL):
            for i, _ in enumerate(att_steps(h, 0)):
                if i in (4, 10) and pending_q:
                    qk_tile(xt3, sc, *QSET, pending_q.pop(0), psB, "psB")
        for h in pending_q:
            qk_tile(xt3, sc, *QSET, h, psB, "psB")

        # ---- q block 1: attention interleaved with q-block-0 out-proj ----
        dt_iter = iter(range(NDT))
        for h in range(NHL):
            for i, _ in enumerate(att_steps(h, 1)):
                if i % 4 == 3:
                    dt = next(dt_iter, None)
                    if dt is not None:
                        p3_chunk(dt, 0)
        for dt in dt_iter:
            p3_chunk(dt, 0)
        # ---- q block 1 output projection tail ----
        for dt in range(NDT):
            p3_chunk(dt, 1)


def _build_program(loop_n=0, phases=(1, 2, 3)):
    key = ("nc", loop_n, tuple(phases))
    if key in _PROG_CACHE:
        return _PROG_CACHE[key]
    nc = bass.Bass()
    t = {}
    t["xT"] = nc.dram_tensor("xT", [D, S], F16, kind="ExternalInput")
    t["wqT"] = nc.dram_tensor("wqT", [D, IL], F16, kind="ExternalInput")
    t["wkT"] = nc.dram_tensor("wkT", [D, IL], F16, kind="ExternalInput")
    t["wvT"] = nc.dram_tensor("wvT", [D, IL], F16, kind="ExternalInput")
    t["woT"] = nc.dram_tensor("woT", [IL, D], F16, kind="ExternalInput")
    for nm in ("ropeAq", "ropeBq", "ropeAk", "ropeBk"):
        t[nm] = nc.dram_tensor(nm, [P, S], F16, kind="ExternalInput")
    t["pswap"] = nc.dram_tensor("pswap", [P, P], F16, kind="ExternalInput")
    t["outT"] = nc.dram_tensor("outT", [D, S], BF16, kind="ExternalOutput")

    with tile.TileContext(nc) as tc:
        if loop_n:
            with tc.For_i(0, loop_n):
                _emit(nc, tc, t, phases)
        else:
            _emit(nc, tc, t, phases)
    _split_multi_waits(nc)
    _PROG_CACHE[key] = nc
    return nc


def _prep_in_maps(x, rope_emb, Wq, Wk, Wv, Wo, q_norm_w, k_norm_w):
    x = np.asarray(x, np.float32)
    F = np.asarray(rope_emb, np.float32)[:, 0]          # [S, 64, 2, 2]
    A0 = np.concatenate([F[:, :, 0, 0], F[:, :, 1, 1]], axis=-1)  # [S, 128]
    B0 = np.concatenate([F[:, :, 0, 1], F[:, :, 1, 0]], axis=-1)  # [S, 128]

    def rope_consts(w):
        w = np.asarray(w, np.float32)
        w_sw = np.concatenate([w[64:], w[:64]])
        A = np.ascontiguousarray((A0 * w[None, :]).T)    # [128, S]
        B = np.ascontiguousarray((B0 * w_sw[None, :]).T)
        return A.astype(np.float16), B.astype(np.float16)

    Aq, Bq = rope_consts(q_norm_w)
    Ak, Bk = rope_consts(k_norm_w)
    pswap = np.zeros((P, P), np.float16)
    for d in range(P):
        pswap[(d + 64) % P, d] = 1.0

    bf = np.float16
    xT = [np.ascontiguousarray(x[b].T).astype(bf) for b in range(x.shape[0])]
    Wq = np.asarray(Wq, np.float32)
    Wk = np.asarray(Wk, np.float32)
    Wv = np.asarray(Wv, np.float32)
    Wo = np.asarray(Wo, np.float32)

    in_maps = []
    for c in range(N_CORES):
        b, hg = divmod(c, NH // NHL)
        sl = slice(hg * IL, (hg + 1) * IL)
        in_maps.append({
            "xT": xT[b],
            "wqT": np.ascontiguousarray(Wq[sl, :].T).astype(bf),
            "wkT": np.ascontiguousarray(Wk[sl, :].T).astype(bf),
            "wvT": np.ascontiguousarray(Wv[sl, :].T).astype(bf),
            "woT": np.ascontiguousarray(Wo[:, sl].T).astype(bf),
            "ropeAq": Aq, "ropeBq": Bq, "ropeAk": Ak, "ropeBk": Bk,
            "pswap": pswap,
        })
    return in_maps


def kernel(x, rope_emb, Wq, Wk, Wv, Wo, q_norm_w, k_norm_w, _trace=False):
    nc = _build_program()
    in_maps = _prep_in_maps(x, rope_emb, Wq, Wk, Wv, Wo, q_norm_w, k_norm_w)
    res = run_bass_kernel_spmd(nc, in_maps, core_ids=list(range(N_CORES)),
                               trace=_trace)
    out = np.empty((2, S, D), np.float32)
    for b in range(2):
        acc = res.results[4 * b]["outT"].astype(np.float32)
        for hg in range(1, 4):
            acc += res.results[4 * b + hg]["outT"].astype(np.float32)
        out[b] = acc.T
    if _trace:
        kernel.last_exec_time_ns = res.exec_time_ns
        kernel.last_results = res
    return out
